# revision 57
# baseline (speedup 1.0000x reference)
"""AttnUpBlock2D Trainium2 kernel.

Pipeline per sample: bilinear up2 (align_corners) -> conv3x3(256->128)+BN+lrelu
-> conv3x3+BN+lrelu -> conv3x3+BN+lrelu -> self-attention (C=128, N=4096)
-> +identity -> lrelu.

Sharding: 8 cores = 4 samples x 2 spatial halves (32 of 64 output rows each).
Convs computed locally with halo rows (host ships pre-gathered, zero-padded
upsample operands). The attention needs the full feature map, so the two
cores of a sample AllGather their conv outputs (bf16; partner recovered
exactly as rank0+rank1-own), then each computes attention for its own 2048
query positions.

Schedule highlights:
- upsample blocks interleave with conv_up row-groups so the PE starts on the
  first 8 interpolated rows; q/k/vT projections interleave with conv-r1.
- two-phase attention: every query quarter's LOCAL-key half is queued ahead
  of the first collective-dependent instruction (partials spilled to SBUF),
  so the exchange is fully hidden; the remote half then normalizes against
  the spilled partials.
- attention inner loop is software-pipelined one j-pair deep (S^T+exp for
  pair i, O matmuls for pair i-1) so the PE never waits on Act/DVE; exp runs
  once per j-pair; softmax row sums accumulate on the DVE across two pairs
  (bf16) before one ones-matmul per quad.
- exp output / v^T / exchange run in bf16 (range-safe: bf16 keeps the f32
  exponent); everything else float32r (fast fp32, ~1.2e-4 rounding), PSUM
  f32. Softmax uses a constant exp-shift (exact for any constant) so no
  row-max pass is needed.
"""

import os
import numpy as np

import concourse.bass as bass
import concourse.bacc as bacc
import concourse.tile as tile
from concourse import mybir
from concourse.bass_utils import run_bass_kernel_spmd

f32 = mybir.dt.float32
f32r = mybir.dt.float32r
bf16 = mybir.dt.bfloat16

B, CIN, C, HIN, WIN = 4, 256, 128, 32, 32
H, W = 64, 64                  # upsampled
N = H * W                      # 4096 positions per sample
HH = 32                        # rows per core (half)
M = HH * W                     # 2048 own positions per core
D = C // 2                     # 64 qk dim
EPS = 1e-5
ALPHA = 0.2
SHIFT = 40.0                   # exp shift; observed logit max ~53.6, shift keeps exp args <= ~14

UPR = 38                       # upsample rows computed per core ([-3, 35) rel to base)
Y0R = 36                       # y0 rows ([-2, 34))
Y1R = 34                       # y1 rows ([-1, 33))
WB = W + 2                     # padded width
N_CORES = 8
GROUPS = [[0, 1], [2, 3], [4, 5], [6, 7]]

_PROGRAM = None
LAST_RUN = None                # BassKernelResults of the most recent kernel() call


def _row_groups(nrows, step=8):
    out = []
    r = 0
    while r < nrows:
        out.append((r, min(step, nrows - r)))
        r += step
    return out


def build_program(reps=1):
    """reps>1 repeats the whole compute body (same output) — used only for
    differential hardware timing; the graded path uses reps=1."""
    global _PROGRAM
    if _PROGRAM is not None and reps == 1:
        return _PROGRAM

    nc = bacc.Bacc("TRN2", target_bir_lowering=False, debug=False,
                   num_devices=N_CORES)

    def din(name, shape):
        return nc.dram_tensor(name, list(shape), f32, kind="ExternalInput").ap()

    XWR = 22                         # x-row window per core (zero-padded)
    # small per-partition weights are packed host-side into two tensors so
    # the critical early DMA path has few HWDGE dispatches (~650ns each):
    # upw = [upm | w1a | w1bo | w1be]   (needed by the first upsample block)
    # smw = [b0|b1|b2|bq2|bk2|bv|ones | my0 | my1]
    UPW = 2 * UPR + 2 * (WIN - 1)
    SMW = 7 + Y0R + Y1R
    xw_ap = din("xw", [128, 2, XWR, WIN])
    upw_ap = din("upw", [128, UPW])
    smw_ap = din("smw", [128, SMW])
    wu_ap = din("wu", [128, 2, 9, C])
    w0_ap = din("w0", [128, 9, C])
    w1_ap = din("w1", [128, 9, C])
    wqk_ap = din("wqk", [C, 4 * D])      # [Wq.T | Wq.T | Wk.T | Wk.T]
    wv2_ap = din("wv2", [C, 2, C])       # [Wv.T | Wv.T]
    out_ap = nc.dram_tensor("out", [C, HH, W], f32, kind="ExternalOutput").ap()

    # pair exchange runs in bf16 via one AllGather; the partner half is
    # recovered exactly as (rank0 + rank1) - own in f32 arithmetic on the
    # gathered bf16 halves (rank-agnostic SPMD)
    MA = M                       # single exchange covering all 32 rows
    y2_dram = [nc.dram_tensor("y2da", [C, M], bf16).ap()]
    ag_out = [nc.dram_tensor("agouta", [2, C, M], bf16).ap()]

    with tile.TileContext(nc) as tc:
        from contextlib import ExitStack
        for _rep in range(reps):
          with ExitStack() as ctx:
              wp = ctx.enter_context(tc.tile_pool(name=f"wp{_rep}", bufs=1))
              sb = ctx.enter_context(tc.tile_pool(name=f"sb{_rep}", bufs=1))
              pts = ctx.enter_context(tc.tile_pool(name=f"pts{_rep}", bufs=6))
              fin = ctx.enter_context(tc.tile_pool(name=f"fin{_rep}", bufs=2))
              iop = ctx.enter_context(tc.tile_pool(name=f"iop{_rep}", bufs=2))
              # "st" slots ([128, 512] = 1 bank x 4 bufs) serve the conv
              # groups, q/k/vT scratch AND the attention S^T tiles
              psS = ctx.enter_context(tc.tile_pool(name=f"psS{_rep}", bufs=3, space="PSUM"))
              psO = ctx.enter_context(tc.tile_pool(name=f"psO{_rep}", bufs=2, space="PSUM"))

              # ---- input window first (upsample is the pipeline head) ----
              # conv-phase-only tensors live in their own pool, exited before
              # the attention tiles allocate, so SBUF is reused
              convp_cm = tc.tile_pool(name=f"convp{_rep}", bufs=1)
              convp = convp_cm.__enter__()
              UP_BLOCKS = ((0, 8), (8, 18), (18, 28), (28, UPR))
              up = convp.tile([128, 2, UPR, WB], f32r)
              upp_cm = tc.tile_pool(name=f"upp{_rep}", bufs=1)
              upp = upp_cm.__enter__()
              xw = upp.tile([128, 2, XWR, WIN], f32r)
              nc.sync.dma_start(out=xw[:, :, 0:8, :],
                                in_=xw_ap.bitcast(f32r)[:, :, 0:8, :])
              nc.sync.dma_start(out=xw[:, :, 8:, :],
                                in_=xw_ap.bitcast(f32r)[:, :, 8:, :])
              if _rep > 0:
                  # serialize reps for differential timing: perturb xw by
                  # 0 * (previous rep's last output block) so rep N+1 can't
                  # start before rep N's final DMA lands
                  dummy = upp.tile([C, 2, WIN], f32)
                  nc.sync.dma_start(
                      out=dummy,
                      in_=out_ap[:, HH - 1:HH, :].rearrange(
                          "c r (a w) -> c (r a) w", a=2))
                  nc.vector.tensor_scalar_mul(out=dummy, in0=dummy, scalar1=0.0)
                  nc.vector.tensor_tensor(out=xw[:, :, 0, :],
                                          in0=xw[:, :, 0, :].bitcast(f32),
                                          in1=dummy,
                                          op=mybir.AluOpType.add)
              # ---- constants / weights; DMA order is latency-critical:
              # upw gates the first upsample block, wu gates conv group 0
              upw = wp.tile([128, UPW], f32)
              nc.sync.dma_start(out=upw, in_=upw_ap)
              upm = upw[:, 0:UPR]
              w1a = upw[:, UPR:2 * UPR]
              w1bo = upw[:, 2 * UPR:2 * UPR + WIN - 1]
              w1be = upw[:, 2 * UPR + WIN - 1:UPW]
              wu = wp.tile([128, 2, 9, C], f32r)
              nc.sync.dma_start(out=wu, in_=wu_ap.bitcast(f32r))
              smw = wp.tile([128, SMW], f32)
              nc.sync.dma_start(out=smw, in_=smw_ap)
              b0 = smw[:, 0:1]
              b1 = smw[:, 1:2]
              b2 = smw[:, 2:3]
              bq2 = smw[:, 3:4]
              bk2 = smw[:, 4:5]
              bv = smw[:, 5:6]
              my0 = smw[:, 7:7 + Y0R]
              my1 = smw[:, 7 + Y0R:SMW]
              w0 = wp.tile([128, 9, C], f32r)
              nc.sync.dma_start(out=w0, in_=w0_ap.bitcast(f32r))
              w1t = wp.tile([128, 9, C], f32r)
              nc.sync.dma_start(out=w1t, in_=w1_ap.bitcast(f32r))
              # q/k projection weights duplicated side by side: one matmul
              # writes q (partitions 0..63) and its copy (64..127), which the
              # paired S^T matmuls read directly — no dup copy needed.
              wqk = wp.tile([C, 4 * D], f32r)
              nc.sync.dma_start(out=wqk, in_=wqk_ap.bitcast(f32r))
              wq2 = wqk[:, 0:2 * D]
              wk2 = wqk[:, 2 * D:4 * D]
              # wvt duplicated side by side: the vT matmuls then have a
              # 256-wide moving operand (1 cyc/row in f32r vs 4 at 128)
              wvt2 = wp.tile([C, 2, C], f32r)
              nc.sync.dma_start(out=wvt2, in_=wv2_ap.bitcast(f32r))
              onesc = wp.tile([128, 1], f32r)
              nc.vector.memset(onesc.bitcast(f32), 1.0)
              onesb = wp.tile([128, 1], bf16)
              nc.vector.tensor_copy(onesb, onesc.bitcast(f32))
              zbias = wp.tile([C, 1], f32)
              nc.vector.memset(zbias, 0.0)
              alpha = wp.tile([C, 1], f32)
              nc.vector.memset(alpha, ALPHA)
              nshift = wp.tile([128, 1], f32)
              nc.vector.memset(nshift, -SHIFT)

              # ---- conv_up emission helper (interleaved with upsample
              # blocks below so the PE starts as soon as the first 10
              # upsampled rows land instead of after the full window) ----
              y0 = convp.tile([C, Y0R, WB], f32r)
              nc.vector.memset(y0[:, :, 0:1].bitcast(f32), 0.0)
              nc.vector.memset(y0[:, :, WB - 1:WB].bitcast(f32), 0.0)

              def emit_conv_up_group(u0, nr):
                  pt = psS.tile([C, nr * W], f32, tag="st")
                  first = True
                  for dy in range(3):
                      for dx in range(3):
                          k = 3 * dy + dx
                          for ch in range(2):
                              nc.tensor.matmul(
                                  pt, wu[:, ch, k, :],
                                  up[:, ch, u0 + dy:u0 + dy + nr, dx:dx + W],
                                  start=first, stop=(k == 8 and ch == 1))
                              first = False
                  nc.scalar.activation(out=y0[:, u0:u0 + nr, 1:1 + W],
                                       in_=pt.rearrange("p (r w) -> p r w", r=nr),
                                       func=mybir.ActivationFunctionType.Prelu,
                                       bias=b0, scale=1.0, alpha=alpha)
                  # halo-row masks as soon as the owning group lands, so the
                  # first conv-r0 group isn't gated on the whole y0 loop
                  for rr in (0, Y0R - 2):
                      if u0 <= rr < u0 + nr:
                          nc.vector.tensor_tensor(
                              out=y0[:, rr:rr + 2, 1:1 + W],
                              in0=y0[:, rr:rr + 2, 1:1 + W].bitcast(f32),
                              in1=bass.AP(tensor=my0.tensor,
                                          offset=my0.offset + rr,
                                          ap=[my0.ap[0], [1, 2], [0, W]]),
                              op=mybir.AluOpType.mult)

              # 6-row first group so conv can start after an 8-row first
              # upsample block (ap 384 >= 256 keeps f32r at full rate)
              cg_list = [(0, 6), (6, 8), (14, 8), (22, 8), (30, 6)]
              cg_next = 0

              # ---- upsample ----
              # Row interp: output rows t=2a and 2a+1 both read x-window
              # rows a, a+1 (align-corners grid: i0(j) = (j-1)//2, and the
              # row phase is identical for both halves since base is
              # even); per-t weights w1a; out-of-image rows zeroed by upm.
              # per-block row-interp scratch (rotating) instead of the full
              # 38-row window: saves ~14KB/partition of SBUF, and lets the
              # first conv group start as soon as block 0 is interpolated
              ublk_cm = tc.tile_pool(name=f"ublk{_rep}", bufs=2)
              ublk = ublk_cm.__enter__()
              nc.vector.memset(up[:, :, :, 0:1].bitcast(f32), 0.0)
              nc.vector.memset(up[:, :, :, WB - 1:WB].bitcast(f32), 0.0)
              for (r0b, r1b) in UP_BLOCKS:
                  nr = r1b - r0b
                  a0 = r0b // 2
                  na = nr // 2 + (nr % 2)
                  drb = ublk.tile([128, 2, na, WIN], f32r, name="drb")
                  nc.vector.tensor_tensor(
                      out=drb,
                      in0=xw[:, :, a0 + 1:a0 + 1 + na, :].bitcast(f32),
                      in1=xw[:, :, a0:a0 + na, :].bitcast(f32),
                      op=mybir.AluOpType.subtract)
                  xrk = ublk.tile([128, 2, nr, WIN], f32r, name="xrk")
                  # DVE APs allow at most 3 free dims, so the paired-row
                  # gather runs per channel-chunk
                  w1a_b = bass.AP(tensor=w1a.tensor,
                                  offset=w1a.offset + r0b,
                                  ap=[w1a.ap[0], [1, nr], [0, WIN]])
                  upm_b = bass.AP(tensor=upm.tensor,
                                  offset=upm.offset + r0b,
                                  ap=[upm.ap[0], [1, nr], [0, WIN]])
                  for ch in range(2):
                      def _pairs(t, row_stride, chunk_stride):
                          return bass.AP(
                              tensor=t.tensor,
                              offset=t.offset + ch * chunk_stride
                              + a0 * row_stride,
                              ap=[t.ap[0], [row_stride, na], [0, 2],
                                  [1, WIN]])
                      drf = drb.bitcast(f32)
                      drv = bass.AP(
                          tensor=drf.tensor,
                          offset=drf.offset + ch * (na * WIN),
                          ap=[drf.ap[0], [WIN, na], [0, 2], [1, WIN]])
                      xwv = _pairs(xw.bitcast(f32), WIN, XWR * WIN)
                      xrc = xrk[:, ch, :, :]
                      nc.vector.tensor_tensor(out=xrc, in0=drv, in1=w1a_b,
                                              op=mybir.AluOpType.mult)
                      nc.vector.tensor_tensor(out=xrc, in0=xrc.bitcast(f32),
                                              in1=xwv, op=mybir.AluOpType.add)
                      for (mr0, mr1) in ((0, 3), (35, UPR)):
                          lo, hi = max(mr0, r0b), min(mr1, r1b)
                          if lo >= hi:
                              continue
                          upm_e = bass.AP(tensor=upm.tensor,
                                          offset=upm.offset + lo,
                                          ap=[upm.ap[0], [1, hi - lo],
                                              [0, WIN]])
                          xre = xrk[:, ch, lo - r0b:hi - r0b, :]
                          nc.vector.tensor_tensor(out=xre,
                                                  in0=xre.bitcast(f32),
                                                  in1=upm_e,
                                                  op=mybir.AluOpType.mult)
                  dck = ublk.tile([128, 2, nr, WIN - 1], f32r, name="dck")
                  nc.vector.tensor_tensor(out=dck,
                                          in0=xrk[:, :, :, 1:].bitcast(f32),
                                          in1=xrk[:, :, :, :-1].bitcast(f32),
                                          op=mybir.AluOpType.subtract)
                  nc.vector.tensor_copy(up[:, :, r0b:r1b, 1],
                                        xrk[:, :, :, 0].bitcast(f32))
                  nc.vector.tensor_copy(up[:, :, r0b:r1b, 1 + (W - 1)],
                                        xrk[:, :, :, WIN - 1].bitcast(f32))
                  dc_f = dck.bitcast(f32)[:, :, :, 0:WIN - 1]
                  xr_f = xrk.bitcast(f32)[:, :, :, 0:WIN - 1]
                  for (wt, col0) in ((w1bo, 2), (w1be, 3)):
                      out_v = bass.AP(tensor=up.tensor,
                                      offset=up.offset + r0b * WB + col0,
                                      ap=[up.ap[0], up.ap[1], [WB, nr],
                                          [2, WIN - 1]])
                      wt_b = bass.AP(tensor=wt.tensor, offset=wt.offset,
                                     ap=[wt.ap[0], [0, 2], [0, nr],
                                         wt.ap[1]])
                      nc.vector.tensor_tensor(out=out_v, in0=dc_f,
                                              in1=wt_b,
                                              op=mybir.AluOpType.mult)
                      nc.vector.tensor_tensor(out=out_v,
                                              in0=out_v.bitcast(f32),
                                              in1=xr_f,
                                              op=mybir.AluOpType.add)
                  # conv_up groups whose up-row window [u0, u0+nr+2) is now
                  # fully computed
                  while (cg_next < len(cg_list)
                         and cg_list[cg_next][0] + cg_list[cg_next][1] + 2
                         <= r1b):
                      emit_conv_up_group(*cg_list[cg_next])
                      cg_next += 1

              while cg_next < len(cg_list):
                  emit_conv_up_group(*cg_list[cg_next])
                  cg_next += 1

              ublk_cm.__exit__(None, None, None)
              upp_cm.__exit__(None, None, None)

              # mask out-of-image halo rows (data-driven per core); only the
              # first/last two rows can ever be masked, keep the ops tiny


              # ---- conv r0 + bn1 + lrelu -> y1 ----
              y1 = convp.tile([C, Y1R, WB], f32r)
              nc.vector.memset(y1[:, :, 0:1].bitcast(f32), 0.0)
              nc.vector.memset(y1[:, :, WB - 1:WB].bitcast(f32), 0.0)
              for (v0, nr) in _row_groups(Y1R):
                  pt = psS.tile([C, nr * W], f32, tag="st")
                  for dy in range(3):
                      for dx in range(3):
                          k = 3 * dy + dx
                          nc.tensor.matmul(
                              pt, w0[:, k, :],
                              y0[:, v0 + dy:v0 + dy + nr, dx:dx + W],
                              start=(k == 0), stop=(k == 8))
                  nc.scalar.activation(out=y1[:, v0:v0 + nr, 1:1 + W],
                                       in_=pt.rearrange("p (r w) -> p r w", r=nr),
                                       func=mybir.ActivationFunctionType.Prelu,
                                       bias=b1, scale=1.0, alpha=alpha)
                  for rr in (0, Y1R - 1):
                      if v0 <= rr < v0 + nr:
                          nc.vector.tensor_tensor(
                              out=y1[:, rr:rr + 1, 1:1 + W],
                              in0=y1[:, rr:rr + 1, 1:1 + W].bitcast(f32),
                              in1=bass.AP(tensor=my1.tensor,
                                          offset=my1.offset + rr,
                                          ap=[my1.ap[0], [1, 1], [0, W]]),
                              op=mybir.AluOpType.mult)

              # ---- conv r1 + bn2 + lrelu -> y2 (flat [C, M]), with the
              # local q/k/vT projections interleaved per row-group: each
              # 8-row chunk of y2 feeds its q/k/vT chunk immediately, so the
              # PE never idles at the conv->attention transition ----
              y2 = sb.tile([C, HH, W], f32r)
              y2f = y2.rearrange("p r w -> p (r w)")
              # q2: [wq|wq] stationary writes q into partitions 0..63 AND a
              # copy into 64..127 in one matmul, so S^T n-chunk pairs can run
              # as concurrent row-group-packed matmuls.
              q2 = sb.tile([128, M], f32r)
              MJ = M // 128
              k2h = [sb.tile([128, M], f32r, name=f"k2h{h}")
                     for h in range(2)]
              vTh = [sb.tile([128, MJ, C], bf16, name=f"vTh{h}")
                     for h in range(2)]

              def project_chunk_kv(half, src_flat, c0):
                  pk = psS.tile([128, 512], f32, tag="st")
                  nc.tensor.matmul(pk, wk2, src_flat[:, c0:c0 + 512],
                                   start=True, stop=True)
                  nc.scalar.activation(out=k2h[half][:, c0:c0 + 512],
                                       in_=pk,
                                       func=mybir.ActivationFunctionType.Prelu,
                                       bias=bk2, scale=1.0,
                                       alpha=onesc.bitcast(f32))
                  for jj in range(c0 // 128, c0 // 128 + 4):
                      # psO's "po" slots are idle during the conv phase;
                      # borrowing them keeps psS's 4 slots for conv+q/k
                      pv = psO.tile([128, 2, C], f32, tag="po")
                      nc.tensor.matmul(pv,
                                       src_flat[:, jj * 128:(jj + 1) * 128],
                                       wvt2.rearrange("p a b -> p (a b)"),
                                       start=True, stop=True)
                      nc.vector.tensor_copy(vTh[half][:, jj, :],
                                            pv[:, 0, :])

              def emit_exchange(hx):
                  if os.environ.get("KERNEL_NO_COLLECTIVE", "0") == "1":
                      # timing probe only (wrong results): local copies in
                      # place of the pair exchange
                      nc.sync.dma_start(out=ag_out[hx][0], in_=y2_dram[hx])
                      nc.sync.dma_start(out=ag_out[hx][1], in_=y2_dram[hx])
                  else:
                      nc.gpsimd.collective_compute(
                          "AllGather", mybir.AluOpType.bypass,
                          replica_groups=GROUPS,
                          ins=[y2_dram[hx].opt()],
                          outs=[ag_out[hx].opt()])

              def project_chunk_q(c0):
                  pq = psS.tile([128, 512], f32, tag="st")
                  nc.tensor.matmul(pq, wq2, y2f[:, c0:c0 + 512],
                                   start=True, stop=True)
                  nc.scalar.activation(out=q2[:, c0:c0 + 512], in_=pq,
                                       func=mybir.ActivationFunctionType.Prelu,
                                       bias=bq2, scale=1.0,
                                       alpha=onesc.bitcast(f32))

              # projections lag the conv groups by one so the PE never
              # waits on the freshly-written y2 chunk's Prelu
              pending = []
              for (z0, nr) in _row_groups(HH):
                  pt = psS.tile([C, nr * W], f32, tag="st")
                  for dy in range(3):
                      for dx in range(3):
                          k = 3 * dy + dx
                          nc.tensor.matmul(
                              pt, w1t[:, k, :],
                              y1[:, z0 + dy:z0 + dy + nr, dx:dx + W],
                              start=(k == 0), stop=(k == 8))
                  nc.scalar.activation(out=y2[:, z0:z0 + nr, :],
                                       in_=pt.rearrange("p (r w) -> p r w", r=nr),
                                       func=mybir.ActivationFunctionType.Prelu,
                                       bias=b2, scale=1.0, alpha=alpha)
                  y2h = iop.tile([C, nr, W], bf16, tag="y2h")
                  nc.vector.tensor_copy(y2h, y2[:, z0:z0 + nr, :].bitcast(f32))
                  ho = z0 * W
                  nc.sync.dma_start(
                      out=y2_dram[0][:, ho:ho + nr * W],
                      in_=y2h.rearrange("p r w -> p (r w)"))
                  for c0 in pending:
                      project_chunk_q(c0)
                      project_chunk_kv(0, y2f, c0)
                  pending = [z0 * W]
              for c0 in pending:
                  project_chunk_q(c0)
                  project_chunk_kv(0, y2f, c0)

              emit_exchange(0)

              NJ = N // 128
              NJ2 = NJ // 2

              def attn_quarter(ms, pO, psums, j_first, j_last):
                  # each call accumulates j-chunks [j_first, j_last] into a
                  # fresh PSUM tile which is then folded into the SBUF
                  # partials (oloc/sloc)
                  mlo = ms * 512
                  # software-pipelined by one j-pair: iteration i emits
                  # S^T+exp+pair-sum for pair i, then the O matmuls for pair
                  # i-1 — so the PE never sits behind a fresh exp (Act) or
                  # pair-sum (DVE) in its own queue. Row sums accumulate on
                  # the DVE across TWO pairs (a quad) before one ones-matmul,
                  # halving the PE's sum traffic again.
                  pending = None
                  quad = []

                  def flush(pend):
                      j0p, pt_p = pend
                      for u in range(2):
                          j = j0p + u
                          nc.tensor.matmul(pO, vTh[j // MJ][:, j % MJ, :],
                                           pt_p[:, u, :],
                                           start=(j == j_first),
                                           stop=(j == j_last))

                  for j0 in range(j_first, j_last + 1, 2):
                      pS = psS.tile([128, 2, 512], f32, tag="st")
                      for u in range(2):
                          j = j0 + u
                          bp0 = u * D
                          nc.tensor.matmul(pS[:, u, :],
                                           k2h[j // MJ][bp0:bp0 + D,
                                                        (j % MJ) * 128:
                                                        (j % MJ + 1) * 128],
                                           q2[bp0:bp0 + D, mlo:mlo + 512],
                                           start=True, stop=True,
                                           tile_position=(bp0, 0))
                      # one exp per j-pair: the Act engine paces the
                      # attention, and halving its instruction count saves
                      # ~180ns of fixed access latency per pair
                      pt = pts.tile([128, 2, 512], bf16, tag="pt")
                      nc.scalar.activation(
                          out=pt, in_=pS,
                          func=mybir.ActivationFunctionType.Exp,
                          bias=nshift, scale=1.0)
                      ptp = pts.tile([128, 512], bf16, tag="ptp")
                      nc.vector.tensor_tensor(out=ptp, in0=pt[:, 0, :],
                                              in1=pt[:, 1, :],
                                              op=mybir.AluOpType.add)
                      quad.append((j0, ptp))
                      if pending is not None:
                          flush(pending)
                      pending = (j0, pt)
                      # quad sum emitted after the lagged O matmuls so the
                      # PE queue head never waits on the DVE quad-add
                      if len(quad) == 2 or j0 == j_last - 1:
                          if len(quad) == 2:
                              nc.vector.tensor_tensor(
                                  out=quad[1][1], in0=quad[0][1],
                                  in1=quad[1][1], op=mybir.AluOpType.add)
                          nc.tensor.matmul(psums, onesb, quad[-1][1],
                                           start=(quad[0][0] == j_first),
                                           stop=(j0 == j_last - 1))
                          quad = []
                  flush(pending)

              # SBUF spill slots for the local-half partial O and row sums:
              # frees all PSUM while the collective is in flight so every
              # quarter's local half can queue ahead of it
              oloc = sb.tile([C, 4, 512], f32)
              sloc = sb.tile([1, 4, 512], bf16)

              def attn_finish(ms, pO, psums, split=1):
                  # split=2 pipelines the tail chain in 256-column halves so
                  # the last quarter's finish drains faster after the final
                  # O matmul. The (pO + oloc) add runs in parallel with the
                  # sums -> recip -> broadcast chain; the broadcast is a PE
                  # matmul (ones-row stationary) since the PE is idle here,
                  # and the final lrelu runs on the (also idle) Act engine.
                  cw = 512 // split
                  for s in range(split):
                      lo = s * cw
                      mlo = ms * 512 + lo
                      onorm = fin.tile([C, cw], f32, tag="on", name="onorm")
                      nc.vector.tensor_tensor(out=onorm,
                                              in0=pO[:, lo:lo + cw],
                                              in1=oloc[:, ms, lo:lo + cw],
                                              op=mybir.AluOpType.add)
                      sums = fin.tile([1, cw], f32, tag="sm", name="sums")
                      nc.vector.tensor_tensor(out=sums,
                                              in0=psums[:, lo:lo + cw],
                                              in1=sloc[:, ms, lo:lo + cw],
                                              op=mybir.AluOpType.add)
                      recip = fin.tile([1, cw], f32, tag="rc", name="recip")
                      with nc.allow_low_precision(reason="softmax denominator"):
                          nc.vector.reciprocal(out=recip, in_=sums)
                      rbs = fin.tile([128, cw], f32, tag="rb", name="rbs")
                      nc.gpsimd.partition_broadcast(rbs, recip)
                      nc.vector.tensor_tensor(out=onorm, in0=onorm, in1=rbs,
                                              op=mybir.AluOpType.mult)
                      nc.vector.tensor_tensor(out=onorm, in0=onorm,
                                              in1=y20[:, mlo:mlo + cw],
                                              op=mybir.AluOpType.add)
                      osb = fin.tile([C, cw], f32, tag="ob", name="osb")
                      nc.scalar.activation(
                          out=osb, in_=onorm,
                          func=mybir.ActivationFunctionType.Prelu,
                          bias=zbias, scale=1.0, alpha=alpha)
                      r0 = ms * 8 + s * (8 // split)
                      nc.sync.dma_start(
                          out=out_ap[:, r0:r0 + 8 // split, :],
                          in_=osb.rearrange("p (r w) -> p r w", r=8 // split))

              # ALL local-half attention queues ahead of anything that
              # needs the collectives; PSUM partials spill-accumulate into
              # SBUF (oloc/sloc) after each key-quarter
              def spill_quarter(ms, j_first, j_last):
                  pO = psO.tile([C, 512], f32, tag="po")
                  psums = psO.tile([1, 512], f32, tag="po", name="psums")
                  attn_quarter(ms, pO, psums, j_first, j_last)
                  if j_first == 0:
                      nc.vector.tensor_copy(oloc[:, ms, :], pO)
                      nc.vector.tensor_copy(sloc[:, ms, :], psums)
                  else:
                      nc.vector.tensor_tensor(out=oloc[:, ms, :],
                                              in0=pO, in1=oloc[:, ms, :],
                                              op=mybir.AluOpType.add)
                      nc.vector.tensor_tensor(out=sloc[:, ms, :],
                                              in0=psums, in1=sloc[:, ms, :],
                                              op=mybir.AluOpType.add)

              for ms in range(4):
                  spill_quarter(ms, 0, 15)

              # residual sum (attention adds xf + identity): y20 = y2 + y0_core
              # + bv (the attention v-bias collapses to a constant add since
              # softmax rows sum to 1), so the final lrelu is bias-free and
              # can run on the DVE instead of the busy scalar engine.
              # Emitted after the local quarters: it is only needed by the
              # finishes, so it must not block the S^T bias adds on the DVE.
              y20 = sb.tile([C, M], f32)
              nc.vector.tensor_tensor(
                  out=y20.rearrange("p (r w) -> p r w", r=HH),
                  in0=y2, in1=y0[:, 2:2 + HH, 1:1 + W].bitcast(f32),
                  op=mybir.AluOpType.add)
              bv_b = bass.AP(tensor=bv.tensor, offset=bv.offset,
                             ap=[bv.ap[0], [0, M]])
              nc.vector.tensor_tensor(out=y20, in0=y20, in1=bv_b,
                                      op=mybir.AluOpType.add)
              convp_cm.__exit__(None, None, None)

              # partner half from the AllGather: xrem = (h0 + h1) - own,
              # exact in f32 arithmetic on the gathered bf16 halves (rank-
              # agnostic, so the same SPMD program works on both cores).
              # First collective-dependent instructions on the DMA/DVE queues.
              xrem = sb.tile([C, M], f32r)

              def xrem_chunk(c0):
                  hx, ho = 0, c0
                  xh0 = iop.tile([C, 512], bf16, tag="xh", name="xh0")
                  nc.sync.dma_start(out=xh0,
                                    in_=ag_out[hx][0][:, ho:ho + 512])
                  xh1 = iop.tile([C, 512], bf16, tag="xh", name="xh1")
                  nc.sync.dma_start(out=xh1,
                                    in_=ag_out[hx][1][:, ho:ho + 512])
                  nc.vector.tensor_tensor(
                      out=xrem[:, c0:c0 + 512],
                      in0=xh0, in1=xh1, op=mybir.AluOpType.add)
                  nc.vector.tensor_tensor(
                      out=xrem[:, c0:c0 + 512],
                      in0=xrem[:, c0:c0 + 512].bitcast(f32),
                      in1=y2f[:, c0:c0 + 512].bitcast(f32),
                      op=mybir.AluOpType.subtract)

              # remote keys (first collective-dependent work)
              for c0 in (0, 512, 1024, 1536):
                  xrem_chunk(c0)
                  project_chunk_kv(1, xrem, c0)
              for ms in range(4):
                  pO = psO.tile([C, 512], f32, tag="po")
                  psums = psO.tile([1, 512], f32, tag="po", name="psums")
                  attn_quarter(ms, pO, psums, 16, 31)
                  attn_finish(ms, pO, psums, split=2 if ms == 3 else 1)

    nc.compile()
    if reps == 1:
        _PROGRAM = nc
    return nc


def _prep_inputs(x, W_up, b_up, g0, be0, m0, v0, W_r0, g1, be1, m1, v1,
                 W_r1, g2, be2, m2, v2, Wq, bq, Wk, bk, Wv, bv):
    """Build the 8 per-core input maps (host-side sharding/packing only)."""
    x = np.asarray(x, np.float32)

    def fold(wc, scale):
        return (wc * scale[:, None, None, None]).astype(np.float32)

    def pack(wc):  # [co, ci, 3, 3] -> [ci, 9, co]
        return np.ascontiguousarray(
            wc.transpose(1, 2, 3, 0).reshape(wc.shape[1], 9, wc.shape[0]))

    s0 = np.asarray(g0) / np.sqrt(np.asarray(v0) + EPS)
    s1 = np.asarray(g1) / np.sqrt(np.asarray(v1) + EPS)
    s2 = np.asarray(g2) / np.sqrt(np.asarray(v2) + EPS)
    b0f = (np.asarray(b_up) * s0 + np.asarray(be0) - np.asarray(m0) * s0)
    b1f = (np.asarray(be1) - np.asarray(m1) * s1)
    b2f = (np.asarray(be2) - np.asarray(m2) * s2)

    wu_p = pack(fold(np.asarray(W_up), s0))      # [256, 9, 128]
    wu_p = wu_p.reshape(2, 128, 9, C).transpose(1, 0, 2, 3)
    wu_p = np.ascontiguousarray(wu_p, np.float32)
    w0_p = np.ascontiguousarray(pack(fold(np.asarray(W_r0), s1)), np.float32)
    w1_p = np.ascontiguousarray(pack(fold(np.asarray(W_r1), s2)), np.float32)

    co = np.linspace(0.0, HIN - 1.0, H)
    i0 = np.floor(co).astype(np.int64)
    i1 = np.minimum(i0 + 1, HIN - 1)
    wrow = (co - i0).astype(np.float32)
    w1b_col = (co - i0).astype(np.float32)       # same grid for W axis
    w1bo_t = np.broadcast_to(w1b_col[1:63:2][None, :], (128, WIN - 1)).copy()
    w1be_t = np.broadcast_to(w1b_col[2:63:2][None, :], (128, WIN - 1)).copy()

    wqt = np.ascontiguousarray(np.asarray(Wq).T, np.float32)
    wkt = np.ascontiguousarray(np.asarray(Wk).T, np.float32)
    wq2 = np.ascontiguousarray(np.concatenate([wqt, wqt], axis=1), np.float32)
    wk2 = np.ascontiguousarray(np.concatenate([wkt, wkt], axis=1), np.float32)
    wvt = np.ascontiguousarray(np.asarray(Wv).T, np.float32)
    bq2_c = np.concatenate([np.asarray(bq), np.asarray(bq)]).astype(
        np.float32).reshape(2 * D, 1)
    bk2_c = np.concatenate([np.asarray(bk), np.asarray(bk)]).astype(
        np.float32).reshape(2 * D, 1)
    bv_c = np.asarray(bv, np.float32).reshape(C, 1)
    b0c = b0f.astype(np.float32).reshape(C, 1)
    b1c = b1f.astype(np.float32).reshape(C, 1)
    b2c = b2f.astype(np.float32).reshape(C, 1)

    XWR = 22
    in_maps = []
    for core in range(N_CORES):
        s, h = core // 2, core % 2
        base = HH * h
        xs = x[s]                                # [256, 32, 32]
        rlo = base // 2 - 2
        xw = np.zeros((CIN, XWR, WIN), np.float32)
        for r in range(XWR):
            xr_idx = rlo + r
            if 0 <= xr_idx < HIN:
                xw[:, r, :] = xs[:, xr_idx, :]
        xw = np.ascontiguousarray(
            xw.reshape(2, 128, XWR, WIN).transpose(1, 0, 2, 3))
        w1a = np.zeros((UPR,), np.float32)
        upm = np.zeros((UPR,), np.float32)
        for t in range(UPR):
            j = base - 3 + t
            if 0 <= j < H:
                upm[t] = 1.0
                # j==0 is the exact-sample row: through the pair formula
                # x[rel a] + w*(x[rel a+1]-x[rel a]) with w=1 it returns
                # x[rel a+1] = x row 0 exactly
                w1a[t] = 1.0 if j == 0 else wrow[j]
        w1a_t = np.broadcast_to(w1a[None, :], (128, UPR)).copy()
        upm_t = np.broadcast_to(upm[None, :], (128, UPR)).copy()

        my0 = np.ones((Y0R,), np.float32)
        my1 = np.ones((Y1R,), np.float32)
        if h == 0:
            my0[0:2] = 0.0                       # y0 rows -2,-1
            my1[0] = 0.0                         # y1 row -1
        else:
            my0[Y0R - 2:] = 0.0                  # y0 rows 64,65
            my1[Y1R - 1] = 0.0                   # y1 row 64
        upw = np.concatenate([upm_t, w1a_t, w1bo_t, w1be_t],
                             axis=1).astype(np.float32)
        smw = np.concatenate(
            [b0c, b1c, b2c, bq2_c, bk2_c, bv_c,
             np.ones((128, 1), np.float32),
             np.broadcast_to(my0[None, :], (128, Y0R)),
             np.broadcast_to(my1[None, :], (128, Y1R))],
            axis=1).astype(np.float32)
        in_maps.append(dict(
            xw=xw, upw=np.ascontiguousarray(upw),
            smw=np.ascontiguousarray(smw),
            wu=wu_p, w0=w0_p, w1=w1_p,
            wqk=np.ascontiguousarray(
                np.concatenate([wq2, wk2], axis=1)),
            wv2=np.ascontiguousarray(
                np.stack([wvt, wvt], axis=1)),
        ))
    return in_maps


def kernel(**inputs):
    global LAST_RUN
    nc = build_program()
    in_maps = _prep_inputs(**inputs)
    trace = bool(int(os.environ.get("KERNEL_TRACE", "0")))
    try:
        res = run_bass_kernel_spmd(nc, in_maps, list(range(N_CORES)),
                                   trace=trace)
    except ModuleNotFoundError:
        # no NTFF profiling hook in this environment; run without trace
        res = run_bass_kernel_spmd(nc, in_maps, list(range(N_CORES)),
                                   trace=False)
    LAST_RUN = res
    out = np.empty((B, C, H, W), np.float32)
    for core in range(N_CORES):
        s, h = core // 2, core % 2
        out[s, :, HH * h:HH * (h + 1), :] = res.results[core]["out"]
    return out

